# revision 35
# baseline (speedup 1.0000x reference)
"""Trainium2 Bass kernel for nn_AttentionBlock (linear attention + BatchNorm).

Math (per batch, c=256 channels, n=1024 pixels, 8 heads x 64 dims):
  qkv = w_qkv @ x                      [1536, n]
  q   = softmax(q, axis=d) * d^-0.5    (per head, over the 64 head-dims)
  k   = softmax(k, axis=n)             (per head-dim, over pixels)
  ctx = k @ (v/n)^T                    [d, e] per head
  out = ctx^T @ q                      [e, n] per head
  y   = BatchNorm(w_out @ out + b_out) (batch stats over (b, n) per channel)

Key restructure vs the straightforward version: w_out is folded into the
per-batch context first —
  W2[(h d), c] = sum_e ctx_h[d, e] * w_out[c, (h e)]
so the per-pixel work becomes a single projection
  final[c, n]  = sum_hd W2[(hd), c] * qhat[(hd), n],   qhat = expq / Zq
eliminating the [e, n] intermediate entirely.  W2 needs ctx^T, obtained with
8 tiny PE transposes per batch.

Scaling: v is used unscaled and SCALE is not applied, so the kernel's final
tensor is s = n/SCALE = 8192 times the reference pre-BN activation.  BatchNorm
is scale-invariant except for eps, so the kernel uses eps' = eps * s^2.

Sharding: data-parallel over batch across 8 cores (4 batches each); BN batch
stats are combined with a tiny AllReduce (4 floats per channel-partition).
b_out is skipped: BatchNorm's mean subtraction cancels it exactly.
"""

import os
import sys

import numpy as np

for _p in ("/opt/trn_rl_repo", "/root/.axon_site/_ro/trn_rl_repo"):
    if os.path.isdir(_p) and _p not in sys.path:
        sys.path.insert(0, _p)

import concourse.bacc as bacc
import concourse.tile as tile
from concourse import mybir
from concourse.bass_utils import run_bass_kernel_spmd

F32 = mybir.dt.float32
FP16 = mybir.dt.float16
AF = mybir.ActivationFunctionType
ALU = mybir.AluOpType

N_CORES = 8
B = int(os.environ.get("BASS_ATTN_B", "4"))  # batches per core
C = 256          # channels
NPIX = 1024      # pixels (32*32)
H = 8            # heads
D = 64           # head dim
HID = H * D      # 512
NT = NPIX // 128  # 8 n-tiles
CT = C // 128     # 2 c-tiles
QT = HID // 128   # 4 q-tiles
SCALE = D ** -0.5
# kernel-side pre-BN tensor is s=(NPIX/SCALE)x the reference one; eps scales s^2
S_FOLD = NPIX / SCALE                     # 8192
BN_EPS_EFF = 1e-5 * S_FOLD * S_FOLD       # 671.08864


def _emit(tc, x, wqkv, wout, ident, gammab, betab, y):
    nc = tc.nc
    from contextlib import ExitStack
    ctx_stack = ExitStack()
    with ctx_stack:
        const = ctx_stack.enter_context(tc.tile_pool(name="const", bufs=1))
        xin = ctx_stack.enter_context(tc.tile_pool(name="xin", bufs=4))
        kvsb = ctx_stack.enter_context(tc.tile_pool(name="kvsb", bufs=3))
        qpool = ctx_stack.enter_context(tc.tile_pool(name="qpool", bufs=4))
        rpool = ctx_stack.enter_context(tc.tile_pool(name="rpool", bufs=4))
        qhatp = ctx_stack.enter_context(tc.tile_pool(name="qhatp", bufs=5))
        cpool = ctx_stack.enter_context(tc.tile_pool(name="cpool", bufs=5))
        w2pool = ctx_stack.enter_context(tc.tile_pool(name="w2pool", bufs=5))
        fpool = ctx_stack.enter_context(tc.tile_pool(name="fpool", bufs=2 * B))
        # 6 staging bufs so applies never gate on output-transfer completion
        stg = ctx_stack.enter_context(tc.tile_pool(name="stg", bufs=6))
        small = ctx_stack.enter_context(tc.tile_pool(name="small", bufs=8))
        stats_p = ctx_stack.enter_context(tc.tile_pool(name="statsp", bufs=1))
        # PSUM: pbig 4 x [128,512] f32 (4 banks, loose FIFO ring so matmuls
        # never wait on a vector read two allocations back), pctx ring of 4
        # banks shared by ctxu -> transpose -> W2 allocations (same tag)
        pbig = ctx_stack.enter_context(
            tc.tile_pool(name="pbig", bufs=4, space="PSUM"))
        pctx = ctx_stack.enter_context(
            tc.tile_pool(name="pctx", bufs=4, space="PSUM"))
        dpool = ctx_stack.enter_context(
            tc.tile_pool(name="dram", bufs=1, space="DRAM"))

        # ---- constants + batch-0 input ----
        # Each dma_start lands on its own DMA ring (~25-50 GB/s per ring), so
        # the first wave is chopped into ~128KB pieces spread across rings,
        # need-ordered: x h0-halves + wqkv k-pieces first (t=0 kv matmuls),
        # v-pieces next, q-pieces later (q phase runs ~15us in).
        xc_b0 = [xin.tile([128, NPIX], FP16, name="xc") for _ in range(CT)]
        wqkv_sb = [const.tile([128, 3 * HID], FP16, name=f"wqkv{kc}")
                   for kc in range(CT)]
        nc.sync.dma_start(out=xc_b0[0][:, 0:512], in_=x[0, 0:128, 0:512])
        nc.scalar.dma_start(out=wqkv_sb[0][:, HID:2 * HID],
                            in_=wqkv[0:128, HID:2 * HID])
        nc.sync.dma_start(out=xc_b0[1][:, 0:512], in_=x[0, 128:256, 0:512])
        nc.scalar.dma_start(out=wqkv_sb[1][:, HID:2 * HID],
                            in_=wqkv[128:256, HID:2 * HID])
        for kc in range(CT):
            nc.sync.dma_start(out=wqkv_sb[kc][:, 2 * HID:3 * HID],
                              in_=wqkv[128 * kc:128 * (kc + 1), 2 * HID:3 * HID])
        for kc in range(CT):
            nc.sync.dma_start(out=xc_b0[kc][:, 512:1024],
                              in_=x[0, 128 * kc:128 * (kc + 1), 512:1024])
        for kc in range(CT):
            nc.sync.dma_start(out=wqkv_sb[kc][:, 0:HID],
                              in_=wqkv[128 * kc:128 * (kc + 1), 0:HID])
        # dummy collective issued up front: the first AllReduce pays a
        # ~24us ncfw rendezvous; running it early overlaps that with compute
        no_cc_warm = os.environ.get("BASS_ATTN_NO_CC") == "1"
        if not no_cc_warm:
            wrm_i = dpool.tile([128, 1], F32, name="wrm_i")
            wrm_o = dpool.tile([128, 1], F32, name="wrm_o")
            wrm_s = const.tile([128, 1], F32, name="wrm_s")
            nc.vector.memset(wrm_s, 0.0)
            nc.sync.dma_start(out=wrm_i, in_=wrm_s)
            nc.gpsimd.collective_compute(
                "AllReduce", ALU.add,
                replica_groups=[list(range(N_CORES))],
                ins=[wrm_i.opt()], outs=[wrm_o.opt()])
        # wout/ident issue on sync: the scalar queue must reach the first Exp
        # (and its ACT table load) as early as possible
        wout_sb = []
        for k4 in range(HID // 128):
            w = const.tile([128, C], FP16, name=f"wout{k4}")
            nc.sync.dma_start(out=w, in_=wout[128 * k4:128 * (k4 + 1), :])
            wout_sb.append(w)
        id_sb = const.tile([128, D], FP16, name="idsb")
        nc.sync.dma_start(out=id_sb, in_=ident)
        gamma2 = const.tile([128, CT], F32, name="gamma2")
        beta2 = const.tile([128, CT], F32, name="beta2")
        for m in range(CT):
            nc.sync.dma_start(out=gamma2[:, m:m + 1],
                              in_=gammab[128 * m:128 * (m + 1), :])
            nc.sync.dma_start(out=beta2[:, m:m + 1],
                              in_=betab[128 * m:128 * (m + 1), :])
        bmask = const.tile([128, 128], FP16, name="bmask")
        nc.vector.memset(bmask, 0.0)
        nc.vector.memset(bmask[0:64, 0:64], 1.0)
        nc.vector.memset(bmask[64:128, 64:128], 1.0)
        eps_sb = const.tile([128, 1], F32, name="eps")
        nc.vector.memset(eps_sb, BN_EPS_EFF)

        stats_sb = [stats_p.tile([128, 2 * B, 6], F32, name=f"stats{m}")
                    for m in range(CT)]
        final_sb = [[None] * CT for _ in range(B)]

        for b in range(B):
            if b == 0:
                xc = xc_b0
            else:
                xc = []
                for kc in range(CT):
                    xt = xin.tile([128, NPIX], FP16, name="xc")
                    for hf in range(2):
                        nc.sync.dma_start(
                            out=xt[:, 512 * hf:512 * (hf + 1)],
                            in_=x[b, 128 * kc:128 * (kc + 1),
                                  512 * hf:512 * (hf + 1)])
                    xc.append(xt)

            # ---- KV projection + context accumulation over n-chunks ----
            # one PSUM bank per head-pair: only one accumulation group may be
            # open per 2KB zero region per partition
            ctxu = [pctx.tile([128, D + 1], F32, name="ctxu", tag="ring")
                    for _ in range(4)]
            expk_t = [None] * NT
            vx_t = [None] * NT

            def emit_kv(t):
                halves = []
                for half in range(2):  # 0: k-cols, 1: v-cols
                    hp = pbig.tile([128, HID], F32, name="kvp", tag="big")
                    for kc in range(CT):
                        nc.tensor.matmul(
                            hp,
                            lhsT=xc[kc][:, 128 * t:128 * (t + 1)],
                            rhs=wqkv_sb[kc][:, HID + 512 * half:
                                            HID + 512 * (half + 1)],
                            start=(kc == 0), stop=(kc == CT - 1))
                    halves.append(hp)
                expk = kvsb.tile([128, HID], FP16, name="expk")
                nc.scalar.activation(out=expk, in_=halves[0], func=AF.Exp)
                vx = kvsb.tile([128, H, D + 1], FP16, name="vx")
                nc.gpsimd.memset(vx[:, :, D:D + 1], 1.0)
                # gpsimd can't read PSUM; alternate the PSUM->SBUF cast
                # between scalar and vector so neither queue binds at the
                # batch boundaries where kv and q phases overlap
                veng = nc.scalar if t % 2 == 0 else nc.vector
                if veng is nc.scalar:
                    veng.copy(vx[:, :, 0:D],
                              halves[1].rearrange("p (h e) -> p h e", h=H))
                else:
                    veng.tensor_copy(
                        vx[:, :, 0:D],
                        halves[1].rearrange("p (h e) -> p h e", h=H))
                expk_t[t] = expk
                vx_t[t] = vx

            def emit_ctx(t):
                for pr in range(4):
                    for j in range(2):
                        h = 2 * pr + j
                        # skip_group_check: j=0/j=1 share the bank but write
                        # disjoint partition ranges; the sim's zero-region
                        # bookkeeping ignores partition base and would raise.
                        nc.tensor.matmul(
                            ctxu[pr][64 * j:64 * (j + 1), :],
                            lhsT=expk_t[t][:, D * h:D * (h + 1)],
                            rhs=vx_t[t][:, h, :],
                            start=(t == 0), stop=(t == NT - 1),
                            tile_position=(0, 64 * j),
                            skip_group_check=True)

            # software-pipelined 2 deep: kv(t+1) and kv(t+2) are issued before
            # ctx(t) so the PE never waits on the exp of the chunk it is about
            # to contract (4 PSUM halves in flight = exactly the pbig ring)
            emit_kv(0)
            emit_kv(1)
            for t in range(2, NT):
                emit_kv(t)
                emit_ctx(t - 2)
            emit_ctx(NT - 2)
            emit_ctx(NT - 1)

            # ---- ctx normalize (1/Zk), transpose, fold w_out -> W2 ----
            # Emitted before the q phase: the PE transposes/W2 matmuls slot in
            # right after the last ctx matmul, and the scalar-side copies
            # drain while the PE streams the q projection, so nothing in this
            # ladder is on the critical path by the time the final runs.
            cs_pr = []
            for pr in range(4):
                rz = small.tile([128, 1], F32, name="rz")
                nc.vector.reciprocal_approx_fast(out=rz, in_=ctxu[pr][:, D:D + 1])
                cs = cpool.tile([128, D], FP16, name="ctxsb")
                nc.vector.tensor_scalar_mul(cs, in0=ctxu[pr][:, 0:D], scalar1=rz)
                cs_pr.append(cs)
            tp_pr = []
            for pr in range(4):
                tp = pctx.tile([128, D], FP16, name="tp", tag="ring")
                for j in range(2):
                    nc.tensor.matmul(
                        tp[64 * j:64 * (j + 1), :],
                        lhsT=cs_pr[pr][64 * j:64 * (j + 1), :],
                        rhs=id_sb[64 * j:64 * (j + 1), :],
                        is_transpose=True,
                        tile_position=(64 * j, 64 * j),
                        skip_group_check=True)
                tp_pr.append(tp)
            csT = []
            for pr in range(4):
                ct_sb = cpool.tile([128, D], FP16, name="csT")
                nc.scalar.copy(ct_sb, tp_pr[pr])
                csT.append(ct_sb)
            w2_sb = []
            for pr in range(4):
                w2p = pctx.tile([128, C], F32, name="w2p", tag="ring")
                for j in range(2):
                    nc.tensor.matmul(
                        w2p[64 * j:64 * (j + 1), :],
                        lhsT=csT[pr][64 * j:64 * (j + 1), :],
                        rhs=wout_sb[pr][64 * j:64 * (j + 1), :],
                        start=True, stop=True,
                        tile_position=(64 * j, 64 * j),
                        skip_group_check=True)
                w2 = w2pool.tile([128, C], FP16, name="w2sb")
                nc.vector.tensor_copy(w2, w2p)
                w2_sb.append(w2)

            # ---- Q projection, exp, Zq block-broadcast, recip, qhat ----
            # Per-half tiles keep the PSUM ring loose; reciprocal on vector,
            # the qhat multiply on the otherwise-idle pool engine (all-SBUF
            # operands, which pool is allowed to touch).
            qhat = []
            prev = None

            def emit_zq(qt, qph, eqh):
                qh = qhatp.tile([128, NPIX], FP16, name="qhat")
                for nch in range(2):
                    # Zq broadcast overwrites qp (WAR-ordered after exp read)
                    nc.tensor.matmul(
                        qph[nch], lhsT=bmask, rhs=eqh[nch],
                        start=True, stop=True)
                    rb = rpool.tile([128, HID], F32, name="recipb")
                    nc.vector.reciprocal_approx_fast(out=rb, in_=qph[nch])
                    nc.gpsimd.tensor_mul(
                        qh[:, 512 * nch:512 * (nch + 1)], eqh[nch], rb)
                qhat.append(qh)

            for qt in range(QT):
                qph, eqh = [], []
                for nch in range(2):
                    qp = pbig.tile([128, HID], F32, name="qp", tag="big")
                    for kc in range(CT):
                        nc.tensor.matmul(
                            qp,
                            lhsT=wqkv_sb[kc][:, 128 * qt:128 * (qt + 1)],
                            rhs=xc[kc][:, 512 * nch:512 * (nch + 1)],
                            start=(kc == 0), stop=(kc == CT - 1))
                    eq = qpool.tile([128, HID], FP16, name="expq")
                    nc.scalar.activation(out=eq, in_=qp, func=AF.Exp)
                    qph.append(qp)
                    eqh.append(eq)
                if prev is not None:
                    emit_zq(*prev)
                prev = (qt, qph, eqh)
            emit_zq(*prev)

            # ---- final projection + bn stats ----
            for m in range(CT):
                fs = fpool.tile([128, NPIX], FP16, name="final")
                fhs = []
                for nch in range(2):
                    # pctx ring (idle after W2): keeps the pbig ring free so
                    # the next batch's kv matmuls never wait on the final's
                    # PSUM drain
                    fh = pctx.tile([128, HID], F32, name="fp", tag="ring")
                    for k4 in range(HID // 128):
                        nc.tensor.matmul(
                            fh,
                            lhsT=w2_sb[k4][:, 128 * m:128 * (m + 1)],
                            rhs=qhat[k4][:, 512 * nch:512 * (nch + 1)],
                            start=(k4 == 0), stop=(k4 == HID // 128 - 1))
                    # stats read the PSUM half directly, and both stats are
                    # emitted before the casts so the cast never delays the
                    # stats chain that feeds the AllReduce
                    nc.vector.bn_stats(
                        out=stats_sb[m][:, 2 * b + nch, :], in_=fh)
                    fhs.append(fh)
                for nch in range(2):
                    nc.vector.tensor_copy(
                        fs[:, 512 * nch:512 * (nch + 1)], fhs[nch])
                final_sb[b][m] = fs

        # ---- batch-norm: aggregate, all-reduce, normalize, store ----
        ccin = dpool.tile([128, 2 * CT], F32, name="ccin")
        ccout = dpool.tile([128, 2 * CT], F32, name="ccout")
        no_cc = os.environ.get("BASS_ATTN_NO_CC") == "1"  # timing-only builds
        # switch the ACT table to the sqrt set while PE still runs the last
        # final-proj matmuls, so the tail's Sqrt doesn't pay the ~1.3us load.
        # Reading the last batch's final tile (not a constant) keeps the
        # scheduler from hoisting this to the start, which would thrash the
        # table back and forth around the first Exp.
        warm_sq = small.tile([1, 1], F32, name="warmsq")
        nc.scalar.activation(out=warm_sq, in_=final_sb[B - 1][CT - 1][0:1, 0:1],
                             func=AF.Sqrt)
        # stat-major packing: cols [mean_m0, mean_m1, ex2_m0, ex2_m1]
        pk4 = small.tile([128, 2 * CT], F32, name="pk4")
        for m in range(CT):
            mv = small.tile([128, 2], F32, name="mv")
            nc.vector.bn_aggr(out=mv, in_=stats_sb[m])
            nc.vector.tensor_mul(pk4[:, CT + m:CT + m + 1],
                                 mv[:, 0:1], mv[:, 0:1])
            nc.vector.tensor_add(pk4[:, CT + m:CT + m + 1],
                                 pk4[:, CT + m:CT + m + 1], mv[:, 1:2])
            nc.vector.tensor_copy(pk4[:, m:m + 1], mv[:, 0:1])
        nc.vector.tensor_scalar_mul(pk4, in0=pk4, scalar1=1.0 / N_CORES)
        nc.sync.dma_start(out=ccin, in_=pk4)
        if not no_cc:
            nc.gpsimd.collective_compute(
                "AllReduce", ALU.add,
                replica_groups=[list(range(N_CORES))],
                ins=[ccin.opt()], outs=[ccout.opt()])
        gst = small.tile([128, 2 * CT], F32, name="gst")
        nc.sync.dma_start(out=gst, in_=ccout if not no_cc else ccin)
        gmean = gst[:, 0:CT]
        var = small.tile([128, CT], F32, name="var")
        nc.vector.tensor_mul(var, gmean, gmean)
        nc.vector.tensor_sub(var, gst[:, CT:2 * CT], var)
        std = small.tile([128, CT], F32, name="std")
        nc.scalar.activation(out=std, in_=var, func=AF.Sqrt, bias=eps_sb)
        rstd = small.tile([128, CT], F32, name="rstd")
        nc.vector.reciprocal_approx_fast(out=rstd, in_=std)
        rsg = small.tile([128, CT], F32, name="rsg")
        nc.vector.tensor_mul(rsg, rstd, gamma2)
        sh = small.tile([128, CT], F32, name="sh")
        nc.vector.tensor_mul(sh, gmean, rsg)
        nc.vector.tensor_sub(sh, beta2, sh)
        # apply per 512-col half, alternating vector/scalar, and store each
        # half with its own dma_start: every dma_start lands on its own ring
        # (~30 GB/s each), so 16x256KB transfers drain the 4MB output across
        # rings far faster than 8x512KB would
        idx = 0
        for b in range(B):
            for m in range(CT):
                fs = final_sb[b][m]
                yst = stg.tile([128, NPIX], F32, name="yst")
                for nch in range(2):
                    sl = slice(512 * nch, 512 * (nch + 1))
                    if idx % 2 == 0:
                        nc.vector.tensor_scalar(
                            out=yst[:, sl], in0=fs[:, sl],
                            scalar1=rsg[:, m:m + 1], scalar2=sh[:, m:m + 1],
                            op0=ALU.mult, op1=ALU.add)
                        nc.sync.dma_start(
                            out=y[b, 128 * m:128 * (m + 1), sl],
                            in_=yst[:, sl])
                    else:
                        nc.scalar.activation(
                            out=yst[:, sl], in_=fs[:, sl], func=AF.Identity,
                            bias=sh[:, m:m + 1], scale=rsg[:, m:m + 1])
                        nc.scalar.dma_start(
                            out=y[b, 128 * m:128 * (m + 1), sl],
                            in_=yst[:, sl])
                    idx += 1


_CACHE = {}


def _build():
    if "nc" in _CACHE:
        return _CACHE["nc"]
    nc = bacc.Bacc("TRN2", target_bir_lowering=False, debug=False,
                   enable_asserts=True, num_devices=N_CORES)
    x = nc.dram_tensor("x", [B, C, NPIX], FP16, kind="ExternalInput").ap()
    wqkv = nc.dram_tensor("wqkvT", [C, 3 * HID], FP16,
                          kind="ExternalInput").ap()
    wout = nc.dram_tensor("woutT", [HID, C], FP16, kind="ExternalInput").ap()
    ident = nc.dram_tensor("ident", [128, D], FP16, kind="ExternalInput").ap()
    gammab = nc.dram_tensor("gammab", [C, 1], F32, kind="ExternalInput").ap()
    betab = nc.dram_tensor("betab", [C, 1], F32, kind="ExternalInput").ap()
    y = nc.dram_tensor("y", [B, C, NPIX], F32, kind="ExternalOutput").ap()
    with tile.TileContext(nc) as tc:
        _emit(tc, x, wqkv, wout, ident, gammab, betab, y)
    nc.compile()
    _CACHE["nc"] = nc
    return nc


def kernel(x, w_qkv, w_out, b_out, gamma, beta, _trace=False):
    x = np.asarray(x, dtype=np.float32)
    wqkvT = np.ascontiguousarray(np.asarray(w_qkv, np.float16).T)   # [256, 1536]
    woutT = np.ascontiguousarray(np.asarray(w_out, np.float16).T)   # [512, 256]
    gammab = np.ascontiguousarray(np.asarray(gamma, np.float32).reshape(C, 1))
    betab = np.ascontiguousarray(np.asarray(beta, np.float32).reshape(C, 1))
    ident = np.zeros((128, D), dtype=np.float16)
    for p in range(128):
        ident[p, p % D] = 1.0
    # b_out is intentionally unused: BatchNorm's mean subtraction cancels any
    # per-channel constant added before it, exactly.

    btot, c, hh, ww = x.shape
    assert (btot, c, hh * ww) == (B * N_CORES, C, NPIX)
    xf = x.reshape(btot, C, NPIX)

    nc = _build()
    in_maps = []
    for core in range(N_CORES):
        in_maps.append({
            "x": np.ascontiguousarray(xf[B * core:B * (core + 1)]).astype(np.float16),
            "wqkvT": wqkvT,
            "woutT": woutT,
            "ident": ident,
            "gammab": gammab,
            "betab": betab,
        })
    res = run_bass_kernel_spmd(nc, in_maps, core_ids=list(range(N_CORES)),
                               trace=_trace)
    y = np.concatenate([res.results[core]["y"] for core in range(N_CORES)],
                       axis=0)
    out = y.reshape(btot, C, hh, ww).astype(np.float32)
    if _trace:
        kernel.last_result = res
    return out


# revision 39
# speedup vs baseline: 1.0804x; 1.0804x over previous
"""Trainium2 Bass kernel for nn_AttentionBlock (linear attention + BatchNorm).

Math (per batch, c=256 channels, n=1024 pixels, 8 heads x 64 dims):
  qkv = w_qkv @ x                      [1536, n]
  q   = softmax(q, axis=d) * d^-0.5    (per head, over the 64 head-dims)
  k   = softmax(k, axis=n)             (per head-dim, over pixels)
  ctx = k @ (v/n)^T                    [d, e] per head
  out = ctx^T @ q                      [e, n] per head
  y   = BatchNorm(w_out @ out + b_out) (batch stats over (b, n) per channel)

Key restructure vs the straightforward version: w_out is folded into the
per-batch context first —
  W2[(h d), c] = sum_e ctx_h[d, e] * w_out[c, (h e)]
so the per-pixel work becomes a single projection
  final[c, n]  = sum_hd W2[(hd), c] * qhat[(hd), n],   qhat = expq / Zq
eliminating the [e, n] intermediate entirely.  W2 needs ctx^T, obtained with
8 tiny PE transposes per batch.

Scaling: v is used unscaled and SCALE is not applied, so the kernel's final
tensor is s = n/SCALE = 8192 times the reference pre-BN activation.  BatchNorm
is scale-invariant except for eps, so the kernel uses eps' = eps * s^2.

Sharding: data-parallel over batch across 8 cores (4 batches each); BN batch
stats are combined with a tiny AllReduce (4 floats per channel-partition).
b_out is skipped: BatchNorm's mean subtraction cancels it exactly.
"""

import os
import sys

import numpy as np

for _p in ("/opt/trn_rl_repo", "/root/.axon_site/_ro/trn_rl_repo"):
    if os.path.isdir(_p) and _p not in sys.path:
        sys.path.insert(0, _p)

import concourse.bacc as bacc
import concourse.tile as tile
from concourse import mybir
from concourse.bass_utils import run_bass_kernel_spmd

F32 = mybir.dt.float32
FP16 = mybir.dt.float16
AF = mybir.ActivationFunctionType
ALU = mybir.AluOpType

N_CORES = 8
B = int(os.environ.get("BASS_ATTN_B", "4"))  # batches per core
C = 256          # channels
NPIX = 1024      # pixels (32*32)
H = 8            # heads
D = 64           # head dim
HID = H * D      # 512
NT = NPIX // 128  # 8 n-tiles
CT = C // 128     # 2 c-tiles
QT = HID // 128   # 4 q-tiles
SCALE = D ** -0.5
# kernel-side pre-BN tensor is s=(NPIX/SCALE)x the reference one; eps scales s^2
S_FOLD = NPIX / SCALE                     # 8192
BN_EPS_EFF = 1e-5 * S_FOLD * S_FOLD       # 671.08864


def _emit(tc, x, wqkv, wout, ident, gammab, betab, y):
    nc = tc.nc
    from contextlib import ExitStack
    ctx_stack = ExitStack()
    with ctx_stack:
        const = ctx_stack.enter_context(tc.tile_pool(name="const", bufs=1))
        xin = ctx_stack.enter_context(tc.tile_pool(name="xin", bufs=4))
        kvsb = ctx_stack.enter_context(tc.tile_pool(name="kvsb", bufs=5))
        qpool = ctx_stack.enter_context(tc.tile_pool(name="qpool", bufs=4))
        rpool = ctx_stack.enter_context(tc.tile_pool(name="rpool", bufs=4))
        qhatp = ctx_stack.enter_context(tc.tile_pool(name="qhatp", bufs=5))
        cpool = ctx_stack.enter_context(tc.tile_pool(name="cpool", bufs=5))
        w2pool = ctx_stack.enter_context(tc.tile_pool(name="w2pool", bufs=5))
        fpool = ctx_stack.enter_context(tc.tile_pool(name="fpool", bufs=2 * B))
        # 6 staging bufs so applies never gate on output-transfer completion
        stg = ctx_stack.enter_context(tc.tile_pool(name="stg", bufs=6))
        small = ctx_stack.enter_context(tc.tile_pool(name="small", bufs=8))
        stats_p = ctx_stack.enter_context(tc.tile_pool(name="statsp", bufs=1))
        # PSUM: pbig 4 x [128,512] f32 (4 banks, loose FIFO ring so matmuls
        # never wait on a vector read two allocations back), pctx ring of 4
        # banks shared by ctxu -> transpose -> W2 allocations (same tag)
        pbig = ctx_stack.enter_context(
            tc.tile_pool(name="pbig", bufs=4, space="PSUM"))
        pctx = ctx_stack.enter_context(
            tc.tile_pool(name="pctx", bufs=4, space="PSUM"))
        dpool = ctx_stack.enter_context(
            tc.tile_pool(name="dram", bufs=1, space="DRAM"))

        # ---- constants + batch-0 input ----
        # Each dma_start lands on its own DMA ring (~25-50 GB/s per ring), so
        # the first wave is chopped into ~128KB pieces spread across rings,
        # need-ordered: x h0-halves + wqkv k-pieces first (t=0 kv matmuls),
        # v-pieces next, q-pieces later (q phase runs ~15us in).
        xc_b0 = [xin.tile([128, NPIX], FP16, name="xc") for _ in range(CT)]
        wqkv_sb = [const.tile([128, 3 * HID], FP16, name=f"wqkv{kc}")
                   for kc in range(CT)]
        nc.sync.dma_start(out=xc_b0[0][:, 0:512], in_=x[0, 0:128, 0:512])
        nc.scalar.dma_start(out=wqkv_sb[0][:, HID:2 * HID],
                            in_=wqkv[0:128, HID:2 * HID])
        nc.sync.dma_start(out=xc_b0[1][:, 0:512], in_=x[0, 128:256, 0:512])
        nc.scalar.dma_start(out=wqkv_sb[1][:, HID:2 * HID],
                            in_=wqkv[128:256, HID:2 * HID])
        for kc in range(CT):
            nc.sync.dma_start(out=wqkv_sb[kc][:, 2 * HID:3 * HID],
                              in_=wqkv[128 * kc:128 * (kc + 1), 2 * HID:3 * HID])
        for kc in range(CT):
            nc.sync.dma_start(out=xc_b0[kc][:, 512:1024],
                              in_=x[0, 128 * kc:128 * (kc + 1), 512:1024])
        for kc in range(CT):
            nc.sync.dma_start(out=wqkv_sb[kc][:, 0:HID],
                              in_=wqkv[128 * kc:128 * (kc + 1), 0:HID])
        # dummy collective issued up front: the first AllReduce pays a
        # ~24us ncfw rendezvous; running it early overlaps that with compute
        no_cc_warm = os.environ.get("BASS_ATTN_NO_CC") == "1"
        if not no_cc_warm:
            wrm_i = dpool.tile([128, 1], F32, name="wrm_i")
            wrm_o = dpool.tile([128, 1], F32, name="wrm_o")
            wrm_s = const.tile([128, 1], F32, name="wrm_s")
            nc.vector.memset(wrm_s, 0.0)
            nc.sync.dma_start(out=wrm_i, in_=wrm_s)
            nc.gpsimd.collective_compute(
                "AllReduce", ALU.add,
                replica_groups=[list(range(N_CORES))],
                ins=[wrm_i.opt()], outs=[wrm_o.opt()])
        # wout/ident issue on sync: the scalar queue must reach the first Exp
        # (and its ACT table load) as early as possible
        wout_sb = []
        for k4 in range(HID // 128):
            w = const.tile([128, C], FP16, name=f"wout{k4}")
            nc.sync.dma_start(out=w, in_=wout[128 * k4:128 * (k4 + 1), :])
            wout_sb.append(w)
        id_sb = const.tile([128, D], FP16, name="idsb")
        nc.sync.dma_start(out=id_sb, in_=ident)
        gamma2 = const.tile([128, CT], F32, name="gamma2")
        beta2 = const.tile([128, CT], F32, name="beta2")
        for m in range(CT):
            nc.sync.dma_start(out=gamma2[:, m:m + 1],
                              in_=gammab[128 * m:128 * (m + 1), :])
            nc.sync.dma_start(out=beta2[:, m:m + 1],
                              in_=betab[128 * m:128 * (m + 1), :])
        bmask = const.tile([128, 128], FP16, name="bmask")
        nc.vector.memset(bmask, 0.0)
        nc.vector.memset(bmask[0:64, 0:64], 1.0)
        nc.vector.memset(bmask[64:128, 64:128], 1.0)
        eps_sb = const.tile([128, 1], F32, name="eps")
        nc.vector.memset(eps_sb, BN_EPS_EFF)

        stats_sb = [stats_p.tile([128, 2 * B, 6], F32, name=f"stats{m}")
                    for m in range(CT)]
        final_sb = [[None] * CT for _ in range(B)]

        for b in range(B):
            if b == 0:
                xc = xc_b0
            else:
                xc = []
                for kc in range(CT):
                    xt = xin.tile([128, NPIX], FP16, name="xc")
                    for hf in range(2):
                        nc.sync.dma_start(
                            out=xt[:, 512 * hf:512 * (hf + 1)],
                            in_=x[b, 128 * kc:128 * (kc + 1),
                                  512 * hf:512 * (hf + 1)])
                    xc.append(xt)

            # ---- KV projection + context accumulation over n-chunks ----
            # one PSUM bank per head-pair: only one accumulation group may be
            # open per 2KB zero region per partition
            ctxu = [pctx.tile([128, D + 1], F32, name="ctxu", tag="ring")
                    for _ in range(4)]
            expk_t = [None] * NT
            vx_t = [None] * NT

            def emit_kv(t):
                # kc-outer so consecutive matmuls share the same stationary
                # xc slice (both PSUM accumulation groups stay open, one per
                # bank, which is legal)
                halves = [pbig.tile([128, HID], F32, name="kvp", tag="big")
                          for _ in range(2)]
                for kc in range(CT):
                    for half in range(2):  # 0: k-cols, 1: v-cols
                        nc.tensor.matmul(
                            halves[half],
                            lhsT=xc[kc][:, 128 * t:128 * (t + 1)],
                            rhs=wqkv_sb[kc][:, HID + 512 * half:
                                            HID + 512 * (half + 1)],
                            start=(kc == 0), stop=(kc == CT - 1))
                expk = kvsb.tile([128, HID], FP16, name="expk")
                nc.scalar.activation(out=expk, in_=halves[0], func=AF.Exp)
                vx = kvsb.tile([128, H, D + 1], FP16, name="vx")
                nc.gpsimd.memset(vx[:, :, D:D + 1], 1.0)
                # gpsimd can't read PSUM; alternate the PSUM->SBUF cast
                # between scalar and vector so neither queue binds at the
                # batch boundaries where kv and q phases overlap
                veng = nc.scalar if t % 2 == 0 else nc.vector
                if veng is nc.scalar:
                    veng.copy(vx[:, :, 0:D],
                              halves[1].rearrange("p (h e) -> p h e", h=H))
                else:
                    veng.tensor_copy(
                        vx[:, :, 0:D],
                        halves[1].rearrange("p (h e) -> p h e", h=H))
                expk_t[t] = expk
                vx_t[t] = vx

            def emit_ctx(t):
                for pr in range(4):
                    for j in range(2):
                        h = 2 * pr + j
                        # skip_group_check: j=0/j=1 share the bank but write
                        # disjoint partition ranges; the sim's zero-region
                        # bookkeeping ignores partition base and would raise.
                        nc.tensor.matmul(
                            ctxu[pr][64 * j:64 * (j + 1), :],
                            lhsT=expk_t[t][:, D * h:D * (h + 1)],
                            rhs=vx_t[t][:, h, :],
                            start=(t == 0), stop=(t == NT - 1),
                            tile_position=(0, 64 * j),
                            skip_group_check=True)

            # software-pipelined 2 deep: kv(t+1) and kv(t+2) are issued before
            # ctx(t) so the PE never waits on the exp of the chunk it is about
            # to contract (4 PSUM halves in flight = exactly the pbig ring)
            emit_kv(0)
            emit_kv(1)
            for t in range(2, NT):
                emit_kv(t)
                emit_ctx(t - 2)
            emit_ctx(NT - 2)
            emit_ctx(NT - 1)

            # ---- ctx normalize (1/Zk), transpose, fold w_out -> W2 ----
            # Emitted before the q phase: the PE transposes/W2 matmuls slot in
            # right after the last ctx matmul, and the scalar-side copies
            # drain while the PE streams the q projection, so nothing in this
            # ladder is on the critical path by the time the final runs.
            cs_pr = []
            for pr in range(4):
                rz = small.tile([128, 1], F32, name="rz")
                nc.vector.reciprocal_approx_fast(out=rz, in_=ctxu[pr][:, D:D + 1])
                cs = cpool.tile([128, D], FP16, name="ctxsb")
                nc.vector.tensor_scalar_mul(cs, in0=ctxu[pr][:, 0:D], scalar1=rz)
                cs_pr.append(cs)
            tp_pr = []
            for pr in range(4):
                tp = pctx.tile([128, D], FP16, name="tp", tag="ring")
                for j in range(2):
                    nc.tensor.matmul(
                        tp[64 * j:64 * (j + 1), :],
                        lhsT=cs_pr[pr][64 * j:64 * (j + 1), :],
                        rhs=id_sb[64 * j:64 * (j + 1), :],
                        is_transpose=True,
                        tile_position=(64 * j, 64 * j),
                        skip_group_check=True)
                tp_pr.append(tp)
            csT = []
            for pr in range(4):
                ct_sb = cpool.tile([128, D], FP16, name="csT")
                nc.scalar.copy(ct_sb, tp_pr[pr])
                csT.append(ct_sb)
            w2_sb = []
            for pr in range(4):
                w2p = pctx.tile([128, C], F32, name="w2p", tag="ring")
                for j in range(2):
                    nc.tensor.matmul(
                        w2p[64 * j:64 * (j + 1), :],
                        lhsT=csT[pr][64 * j:64 * (j + 1), :],
                        rhs=wout_sb[pr][64 * j:64 * (j + 1), :],
                        start=True, stop=True,
                        tile_position=(64 * j, 64 * j),
                        skip_group_check=True)
                w2 = w2pool.tile([128, C], FP16, name="w2sb")
                nc.vector.tensor_copy(w2, w2p)
                w2_sb.append(w2)

            # ---- Q projection, exp, Zq block-broadcast, recip, qhat ----
            # Per-half tiles keep the PSUM ring loose; reciprocal on vector,
            # the qhat multiply on the otherwise-idle pool engine (all-SBUF
            # operands, which pool is allowed to touch).
            qhat = []
            prev = None

            def emit_zq(qt, qph, eqh):
                qh = qhatp.tile([128, NPIX], FP16, name="qhat")
                for nch in range(2):
                    # Zq broadcast overwrites qp (WAR-ordered after exp read)
                    nc.tensor.matmul(
                        qph[nch], lhsT=bmask, rhs=eqh[nch],
                        start=True, stop=True)
                    rb = rpool.tile([128, HID], F32, name="recipb")
                    nc.vector.reciprocal_approx_fast(out=rb, in_=qph[nch])
                    nc.gpsimd.tensor_mul(
                        qh[:, 512 * nch:512 * (nch + 1)], eqh[nch], rb)
                qhat.append(qh)

            for qt in range(QT):
                # kc-outer: both nch matmuls of a kc share the stationary
                # wqkv slice
                qph = [pbig.tile([128, HID], F32, name="qp", tag="big")
                       for _ in range(2)]
                for kc in range(CT):
                    for nch in range(2):
                        nc.tensor.matmul(
                            qph[nch],
                            lhsT=wqkv_sb[kc][:, 128 * qt:128 * (qt + 1)],
                            rhs=xc[kc][:, 512 * nch:512 * (nch + 1)],
                            start=(kc == 0), stop=(kc == CT - 1))
                eqh = []
                for nch in range(2):
                    eq = qpool.tile([128, HID], FP16, name="expq")
                    nc.scalar.activation(out=eq, in_=qph[nch], func=AF.Exp)
                    eqh.append(eq)
                if prev is not None:
                    emit_zq(*prev)
                prev = (qt, qph, eqh)
            emit_zq(*prev)

            # ---- final projection + bn stats ----
            for m in range(CT):
                fs = fpool.tile([128, NPIX], FP16, name="final")
                # pctx ring (idle after W2): keeps the pbig ring free so the
                # next batch's kv matmuls never wait on the final's PSUM
                # drain.  k4-outer so matmul pairs share the stationary W2
                # slice; both halves accumulate in parallel banks.
                fhs = [pctx.tile([128, HID], F32, name="fp", tag="ring")
                       for _ in range(2)]
                for k4 in range(HID // 128):
                    for nch in range(2):
                        nc.tensor.matmul(
                            fhs[nch],
                            lhsT=w2_sb[k4][:, 128 * m:128 * (m + 1)],
                            rhs=qhat[k4][:, 512 * nch:512 * (nch + 1)],
                            start=(k4 == 0), stop=(k4 == HID // 128 - 1))
                # stats read the PSUM halves directly and are emitted before
                # the casts so the cast never delays the stats chain that
                # feeds the AllReduce
                for nch in range(2):
                    nc.vector.bn_stats(
                        out=stats_sb[m][:, 2 * b + nch, :], in_=fhs[nch])
                for nch in range(2):
                    nc.vector.tensor_copy(
                        fs[:, 512 * nch:512 * (nch + 1)], fhs[nch])
                final_sb[b][m] = fs

        # ---- batch-norm: aggregate, all-reduce, normalize, store ----
        ccin = dpool.tile([128, 2 * CT], F32, name="ccin")
        ccout = dpool.tile([128, 2 * CT], F32, name="ccout")
        no_cc = os.environ.get("BASS_ATTN_NO_CC") == "1"  # timing-only builds
        # switch the ACT table to the sqrt set while PE still runs the last
        # final-proj matmuls, so the tail's Sqrt doesn't pay the ~1.3us load.
        # Reading the last batch's final tile (not a constant) keeps the
        # scheduler from hoisting this to the start, which would thrash the
        # table back and forth around the first Exp.
        warm_sq = small.tile([1, 1], F32, name="warmsq")
        nc.scalar.activation(out=warm_sq, in_=final_sb[B - 1][CT - 1][0:1, 0:1],
                             func=AF.Sqrt)
        # stat-major packing: cols [mean_m0, mean_m1, ex2_m0, ex2_m1]
        pk4 = small.tile([128, 2 * CT], F32, name="pk4")
        for m in range(CT):
            mv = small.tile([128, 2], F32, name="mv")
            nc.vector.bn_aggr(out=mv, in_=stats_sb[m])
            nc.vector.tensor_mul(pk4[:, CT + m:CT + m + 1],
                                 mv[:, 0:1], mv[:, 0:1])
            nc.vector.tensor_add(pk4[:, CT + m:CT + m + 1],
                                 pk4[:, CT + m:CT + m + 1], mv[:, 1:2])
            nc.vector.tensor_copy(pk4[:, m:m + 1], mv[:, 0:1])
        nc.vector.tensor_scalar_mul(pk4, in0=pk4, scalar1=1.0 / N_CORES)
        nc.sync.dma_start(out=ccin, in_=pk4)
        if not no_cc:
            nc.gpsimd.collective_compute(
                "AllReduce", ALU.add,
                replica_groups=[list(range(N_CORES))],
                ins=[ccin.opt()], outs=[ccout.opt()])
        gst = small.tile([128, 2 * CT], F32, name="gst")
        nc.sync.dma_start(out=gst, in_=ccout if not no_cc else ccin)
        gmean = gst[:, 0:CT]
        var = small.tile([128, CT], F32, name="var")
        nc.vector.tensor_mul(var, gmean, gmean)
        nc.vector.tensor_sub(var, gst[:, CT:2 * CT], var)
        std = small.tile([128, CT], F32, name="std")
        nc.scalar.activation(out=std, in_=var, func=AF.Sqrt, bias=eps_sb)
        rstd = small.tile([128, CT], F32, name="rstd")
        nc.vector.reciprocal_approx_fast(out=rstd, in_=std)
        rsg = small.tile([128, CT], F32, name="rsg")
        nc.vector.tensor_mul(rsg, rstd, gamma2)
        sh = small.tile([128, CT], F32, name="sh")
        nc.vector.tensor_mul(sh, gmean, rsg)
        nc.vector.tensor_sub(sh, beta2, sh)
        # apply per 512-col half, alternating vector/scalar, and store each
        # half with its own dma_start: every dma_start lands on its own ring
        # (~30 GB/s each), so 16x256KB transfers drain the 4MB output across
        # rings far faster than 8x512KB would
        idx = 0
        for b in range(B):
            for m in range(CT):
                fs = final_sb[b][m]
                yst = stg.tile([128, NPIX], F32, name="yst")
                for nch in range(2):
                    sl = slice(512 * nch, 512 * (nch + 1))
                    if idx % 2 == 0:
                        nc.vector.tensor_scalar(
                            out=yst[:, sl], in0=fs[:, sl],
                            scalar1=rsg[:, m:m + 1], scalar2=sh[:, m:m + 1],
                            op0=ALU.mult, op1=ALU.add)
                        nc.sync.dma_start(
                            out=y[b, 128 * m:128 * (m + 1), sl],
                            in_=yst[:, sl])
                    else:
                        nc.scalar.activation(
                            out=yst[:, sl], in_=fs[:, sl], func=AF.Identity,
                            bias=sh[:, m:m + 1], scale=rsg[:, m:m + 1])
                        nc.scalar.dma_start(
                            out=y[b, 128 * m:128 * (m + 1), sl],
                            in_=yst[:, sl])
                    idx += 1


_CACHE = {}


def _build():
    if "nc" in _CACHE:
        return _CACHE["nc"]
    nc = bacc.Bacc("TRN2", target_bir_lowering=False, debug=False,
                   enable_asserts=True, num_devices=N_CORES)
    x = nc.dram_tensor("x", [B, C, NPIX], FP16, kind="ExternalInput").ap()
    wqkv = nc.dram_tensor("wqkvT", [C, 3 * HID], FP16,
                          kind="ExternalInput").ap()
    wout = nc.dram_tensor("woutT", [HID, C], FP16, kind="ExternalInput").ap()
    ident = nc.dram_tensor("ident", [128, D], FP16, kind="ExternalInput").ap()
    gammab = nc.dram_tensor("gammab", [C, 1], F32, kind="ExternalInput").ap()
    betab = nc.dram_tensor("betab", [C, 1], F32, kind="ExternalInput").ap()
    y = nc.dram_tensor("y", [B, C, NPIX], F32, kind="ExternalOutput").ap()
    with tile.TileContext(nc) as tc:
        _emit(tc, x, wqkv, wout, ident, gammab, betab, y)
    nc.compile()
    _CACHE["nc"] = nc
    return nc


def kernel(x, w_qkv, w_out, b_out, gamma, beta, _trace=False):
    x = np.asarray(x, dtype=np.float32)
    wqkvT = np.ascontiguousarray(np.asarray(w_qkv, np.float16).T)   # [256, 1536]
    woutT = np.ascontiguousarray(np.asarray(w_out, np.float16).T)   # [512, 256]
    gammab = np.ascontiguousarray(np.asarray(gamma, np.float32).reshape(C, 1))
    betab = np.ascontiguousarray(np.asarray(beta, np.float32).reshape(C, 1))
    ident = np.zeros((128, D), dtype=np.float16)
    for p in range(128):
        ident[p, p % D] = 1.0
    # b_out is intentionally unused: BatchNorm's mean subtraction cancels any
    # per-channel constant added before it, exactly.

    btot, c, hh, ww = x.shape
    assert (btot, c, hh * ww) == (B * N_CORES, C, NPIX)
    xf = x.reshape(btot, C, NPIX)

    nc = _build()
    in_maps = []
    for core in range(N_CORES):
        in_maps.append({
            "x": np.ascontiguousarray(xf[B * core:B * (core + 1)]).astype(np.float16),
            "wqkvT": wqkvT,
            "woutT": woutT,
            "ident": ident,
            "gammab": gammab,
            "betab": betab,
        })
    res = run_bass_kernel_spmd(nc, in_maps, core_ids=list(range(N_CORES)),
                               trace=_trace)
    y = np.concatenate([res.results[core]["y"] for core in range(N_CORES)],
                       axis=0)
    out = y.reshape(btot, C, hh, ww).astype(np.float32)
    if _trace:
        kernel.last_result = res
    return out


# revision 44
# speedup vs baseline: 1.1279x; 1.0440x over previous
"""Trainium2 Bass kernel for nn_AttentionBlock (linear attention + BatchNorm).

Math (per batch, c=256 channels, n=1024 pixels, 8 heads x 64 dims):
  qkv = w_qkv @ x                      [1536, n]
  q   = softmax(q, axis=d) * d^-0.5    (per head, over the 64 head-dims)
  k   = softmax(k, axis=n)             (per head-dim, over pixels)
  ctx = k @ (v/n)^T                    [d, e] per head
  out = ctx^T @ q                      [e, n] per head
  y   = BatchNorm(w_out @ out + b_out) (batch stats over (b, n) per channel)

Key restructure vs the straightforward version: w_out is folded into the
per-batch context first —
  W2[(h d), c] = sum_e ctx_h[d, e] * w_out[c, (h e)]
so the per-pixel work becomes a single projection
  final[c, n]  = sum_hd W2[(hd), c] * qhat[(hd), n],   qhat = expq / Zq
eliminating the [e, n] intermediate entirely.  W2 needs ctx^T, obtained with
8 tiny PE transposes per batch.

Scaling: v is used unscaled and SCALE is not applied, so the kernel's final
tensor is s = n/SCALE = 8192 times the reference pre-BN activation.  BatchNorm
is scale-invariant except for eps, so the kernel uses eps' = eps * s^2.

Sharding: data-parallel over batch across 8 cores (4 batches each); BN batch
stats are combined with a tiny AllReduce (4 floats per channel-partition).
b_out is skipped: BatchNorm's mean subtraction cancels it exactly.
"""

import os
import sys

import numpy as np

for _p in ("/opt/trn_rl_repo", "/root/.axon_site/_ro/trn_rl_repo"):
    if os.path.isdir(_p) and _p not in sys.path:
        sys.path.insert(0, _p)

import concourse.bacc as bacc
import concourse.tile as tile
from concourse import mybir
from concourse.bass_utils import run_bass_kernel_spmd

F32 = mybir.dt.float32
FP16 = mybir.dt.float16
AF = mybir.ActivationFunctionType
ALU = mybir.AluOpType

N_CORES = 8
B = int(os.environ.get("BASS_ATTN_B", "4"))  # batches per core
C = 256          # channels
NPIX = 1024      # pixels (32*32)
H = 8            # heads
D = 64           # head dim
HID = H * D      # 512
NT = NPIX // 128  # 8 n-tiles
CT = C // 128     # 2 c-tiles
QT = HID // 128   # 4 q-tiles
SCALE = D ** -0.5
# kernel-side pre-BN tensor is s=(NPIX/SCALE)x the reference one; eps scales s^2
S_FOLD = NPIX / SCALE                     # 8192
BN_EPS_EFF = 1e-5 * S_FOLD * S_FOLD       # 671.08864


def _emit(tc, x, wqkv, wout, ident, gammab, betab, y):
    nc = tc.nc
    from contextlib import ExitStack
    ctx_stack = ExitStack()
    with ctx_stack:
        const = ctx_stack.enter_context(tc.tile_pool(name="const", bufs=1))
        xin = ctx_stack.enter_context(tc.tile_pool(name="xin", bufs=4))
        kvsb = ctx_stack.enter_context(tc.tile_pool(name="kvsb", bufs=5))
        qpool = ctx_stack.enter_context(tc.tile_pool(name="qpool", bufs=4))
        rpool = ctx_stack.enter_context(tc.tile_pool(name="rpool", bufs=4))
        qhatp = ctx_stack.enter_context(tc.tile_pool(name="qhatp", bufs=5))
        cpool = ctx_stack.enter_context(tc.tile_pool(name="cpool", bufs=5))
        w2pool = ctx_stack.enter_context(tc.tile_pool(name="w2pool", bufs=5))
        fpool = ctx_stack.enter_context(tc.tile_pool(name="fpool", bufs=2 * B))
        # 6 staging bufs so applies never gate on output-transfer completion
        stg = ctx_stack.enter_context(tc.tile_pool(name="stg", bufs=6))
        small = ctx_stack.enter_context(tc.tile_pool(name="small", bufs=8))
        stats_p = ctx_stack.enter_context(tc.tile_pool(name="statsp", bufs=1))
        # PSUM: pbig 4 x [128,512] f32 (4 banks, loose FIFO ring so matmuls
        # never wait on a vector read two allocations back), pctx ring of 4
        # banks shared by ctxu -> transpose -> W2 allocations (same tag)
        pbig = ctx_stack.enter_context(
            tc.tile_pool(name="pbig", bufs=4, space="PSUM"))
        pctx = ctx_stack.enter_context(
            tc.tile_pool(name="pctx", bufs=4, space="PSUM"))
        dpool = ctx_stack.enter_context(
            tc.tile_pool(name="dram", bufs=1, space="DRAM"))

        # ---- constants + batch-0 input ----
        # Each dma_start lands on its own DMA ring (~25-50 GB/s per ring), so
        # the first wave is chopped into ~128KB pieces spread across rings,
        # need-ordered: x h0-halves + wqkv k-pieces first (t=0 kv matmuls),
        # v-pieces next, q-pieces later (q phase runs ~15us in).
        xc_b0 = [xin.tile([128, NPIX], FP16, name="xc") for _ in range(CT)]
        wqkv_sb = [const.tile([128, 3 * HID], FP16, name=f"wqkv{kc}")
                   for kc in range(CT)]
        nc.sync.dma_start(out=xc_b0[0][:, 0:512], in_=x[0, 0:128, 0:512])
        nc.scalar.dma_start(out=wqkv_sb[0][:, HID:2 * HID],
                            in_=wqkv[0:128, HID:2 * HID])
        nc.sync.dma_start(out=xc_b0[1][:, 0:512], in_=x[0, 128:256, 0:512])
        nc.scalar.dma_start(out=wqkv_sb[1][:, HID:2 * HID],
                            in_=wqkv[128:256, HID:2 * HID])
        for kc in range(CT):
            nc.sync.dma_start(out=wqkv_sb[kc][:, 2 * HID:3 * HID],
                              in_=wqkv[128 * kc:128 * (kc + 1), 2 * HID:3 * HID])
        for kc in range(CT):
            nc.sync.dma_start(out=xc_b0[kc][:, 512:1024],
                              in_=x[0, 128 * kc:128 * (kc + 1), 512:1024])
        for kc in range(CT):
            nc.sync.dma_start(out=wqkv_sb[kc][:, 0:HID],
                              in_=wqkv[128 * kc:128 * (kc + 1), 0:HID])
        # dummy collective issued up front: the first AllReduce pays a
        # ~24us ncfw rendezvous; running it early overlaps that with compute
        no_cc_warm = os.environ.get("BASS_ATTN_NO_CC") == "1"
        if not no_cc_warm:
            wrm_i = dpool.tile([128, 1], F32, name="wrm_i")
            wrm_o = dpool.tile([128, 1], F32, name="wrm_o",
                               addr_space="Shared")
            wrm_s = const.tile([128, 1], F32, name="wrm_s")
            nc.vector.memset(wrm_s, 0.0)
            nc.sync.dma_start(out=wrm_i, in_=wrm_s)
            nc.gpsimd.collective_compute(
                "AllReduce", ALU.add,
                replica_groups=[list(range(N_CORES))],
                ins=[wrm_i.opt()], outs=[wrm_o.opt()])
        # wout/ident issue on sync: the scalar queue must reach the first Exp
        # (and its ACT table load) as early as possible
        wout_sb = []
        for k4 in range(HID // 128):
            w = const.tile([128, C], FP16, name=f"wout{k4}")
            nc.sync.dma_start(out=w, in_=wout[128 * k4:128 * (k4 + 1), :])
            wout_sb.append(w)
        id_sb = const.tile([128, D], FP16, name="idsb")
        nc.sync.dma_start(out=id_sb, in_=ident)
        gamma2 = const.tile([128, CT], F32, name="gamma2")
        beta2 = const.tile([128, CT], F32, name="beta2")
        for m in range(CT):
            nc.sync.dma_start(out=gamma2[:, m:m + 1],
                              in_=gammab[128 * m:128 * (m + 1), :])
            nc.sync.dma_start(out=beta2[:, m:m + 1],
                              in_=betab[128 * m:128 * (m + 1), :])
        bmask = const.tile([128, 128], FP16, name="bmask")
        nc.vector.memset(bmask, 0.0)
        nc.vector.memset(bmask[0:64, 0:64], 1.0)
        nc.vector.memset(bmask[64:128, 64:128], 1.0)
        eps_sb = const.tile([128, 1], F32, name="eps")
        nc.vector.memset(eps_sb, BN_EPS_EFF)

        stats_sb = [stats_p.tile([128, 2 * B, 6], F32, name=f"stats{m}")
                    for m in range(CT)]
        final_sb = [[None] * CT for _ in range(B)]
        deferred_cast = []

        for b in range(B):
            if b == 0:
                xc = xc_b0
            else:
                xc = []
                for kc in range(CT):
                    xt = xin.tile([128, NPIX], FP16, name="xc")
                    for hf in range(2):
                        nc.sync.dma_start(
                            out=xt[:, 512 * hf:512 * (hf + 1)],
                            in_=x[b, 128 * kc:128 * (kc + 1),
                                  512 * hf:512 * (hf + 1)])
                    xc.append(xt)

            # ---- KV projection + context accumulation over n-chunks ----
            # one PSUM bank per head-pair: only one accumulation group may be
            # open per 2KB zero region per partition
            ctxu = [pctx.tile([128, D + 1], F32, name="ctxu", tag="ring")
                    for _ in range(4)]
            expk_t = [None] * NT
            vx_t = [None] * NT

            def emit_kv(t):
                # kc-outer so consecutive matmuls share the same stationary
                # xc slice (both PSUM accumulation groups stay open, one per
                # bank, which is legal)
                halves = [pbig.tile([128, HID], F32, name="kvp", tag="big")
                          for _ in range(2)]
                for kc in range(CT):
                    for half in range(2):  # 0: k-cols, 1: v-cols
                        nc.tensor.matmul(
                            halves[half],
                            lhsT=xc[kc][:, 128 * t:128 * (t + 1)],
                            rhs=wqkv_sb[kc][:, HID + 512 * half:
                                            HID + 512 * (half + 1)],
                            start=(kc == 0), stop=(kc == CT - 1))
                expk = kvsb.tile([128, HID], FP16, name="expk")
                nc.scalar.activation(out=expk, in_=halves[0], func=AF.Exp)
                vx = kvsb.tile([128, H, D + 1], FP16, name="vx")
                nc.gpsimd.memset(vx[:, :, D:D + 1], 1.0)
                # gpsimd can't read PSUM; alternate the PSUM->SBUF cast
                # between scalar and vector so neither queue binds at the
                # batch boundaries where kv and q phases overlap
                veng = nc.scalar if t % 2 == 0 else nc.vector
                if veng is nc.scalar:
                    veng.copy(vx[:, :, 0:D],
                              halves[1].rearrange("p (h e) -> p h e", h=H))
                else:
                    veng.tensor_copy(
                        vx[:, :, 0:D],
                        halves[1].rearrange("p (h e) -> p h e", h=H))
                expk_t[t] = expk
                vx_t[t] = vx

            def emit_ctx(t):
                for pr in range(4):
                    for j in range(2):
                        h = 2 * pr + j
                        # skip_group_check: j=0/j=1 share the bank but write
                        # disjoint partition ranges; the sim's zero-region
                        # bookkeeping ignores partition base and would raise.
                        nc.tensor.matmul(
                            ctxu[pr][64 * j:64 * (j + 1), :],
                            lhsT=expk_t[t][:, D * h:D * (h + 1)],
                            rhs=vx_t[t][:, h, :],
                            start=(t == 0), stop=(t == NT - 1),
                            tile_position=(0, 64 * j),
                            skip_group_check=True)

            # software-pipelined 2 deep: kv(t+1) and kv(t+2) are issued before
            # ctx(t) so the PE never waits on the exp of the chunk it is about
            # to contract (4 PSUM halves in flight = exactly the pbig ring)
            emit_kv(0)
            emit_kv(1)
            for t in range(2, NT):
                emit_kv(t)
                emit_ctx(t - 2)
            emit_ctx(NT - 2)
            emit_ctx(NT - 1)

            # ---- ctx normalize (1/Zk), transpose, fold w_out -> W2 ----
            # Emitted before the q phase: the PE transposes/W2 matmuls slot in
            # right after the last ctx matmul, and the scalar-side copies
            # drain while the PE streams the q projection, so nothing in this
            # ladder is on the critical path by the time the final runs.
            cs_pr = []
            for pr in range(4):
                rz = small.tile([128, 1], F32, name="rz")
                nc.vector.reciprocal_approx_fast(out=rz, in_=ctxu[pr][:, D:D + 1])
                cs = cpool.tile([128, D], FP16, name="ctxsb")
                nc.vector.tensor_scalar_mul(cs, in0=ctxu[pr][:, 0:D], scalar1=rz)
                cs_pr.append(cs)
            tp_pr = []
            for pr in range(4):
                tp = pctx.tile([128, D], FP16, name="tp", tag="ring")
                for j in range(2):
                    nc.tensor.matmul(
                        tp[64 * j:64 * (j + 1), :],
                        lhsT=cs_pr[pr][64 * j:64 * (j + 1), :],
                        rhs=id_sb[64 * j:64 * (j + 1), :],
                        is_transpose=True,
                        tile_position=(64 * j, 64 * j),
                        skip_group_check=True)
                tp_pr.append(tp)
            csT = []
            for pr in range(4):
                ct_sb = cpool.tile([128, D], FP16, name="csT")
                nc.scalar.copy(ct_sb, tp_pr[pr])
                csT.append(ct_sb)
            w2_sb = []
            for pr in range(4):
                w2p = pctx.tile([128, C], F32, name="w2p", tag="ring")
                for j in range(2):
                    nc.tensor.matmul(
                        w2p[64 * j:64 * (j + 1), :],
                        lhsT=csT[pr][64 * j:64 * (j + 1), :],
                        rhs=wout_sb[pr][64 * j:64 * (j + 1), :],
                        start=True, stop=True,
                        tile_position=(64 * j, 64 * j),
                        skip_group_check=True)
                w2 = w2pool.tile([128, C], FP16, name="w2sb")
                nc.vector.tensor_copy(w2, w2p)
                w2_sb.append(w2)

            # ---- Q projection, exp, Zq block-broadcast, recip, qhat ----
            # Per-half tiles keep the PSUM ring loose; reciprocal on vector,
            # the qhat multiply on the otherwise-idle pool engine (all-SBUF
            # operands, which pool is allowed to touch).
            qhat = []
            prev = None

            def emit_zq(qt, qph, eqh):
                qh = qhatp.tile([128, NPIX], FP16, name="qhat")
                for nch in range(2):
                    # Zq broadcast overwrites qp (WAR-ordered after exp read)
                    nc.tensor.matmul(
                        qph[nch], lhsT=bmask, rhs=eqh[nch],
                        start=True, stop=True)
                    rb = rpool.tile([128, HID], F32, name="recipb")
                    nc.vector.reciprocal_approx_fast(out=rb, in_=qph[nch])
                    nc.gpsimd.tensor_mul(
                        qh[:, 512 * nch:512 * (nch + 1)], eqh[nch], rb)
                qhat.append(qh)

            for qt in range(QT):
                # kc-outer: both nch matmuls of a kc share the stationary
                # wqkv slice
                qph = [pbig.tile([128, HID], F32, name="qp", tag="big")
                       for _ in range(2)]
                for kc in range(CT):
                    for nch in range(2):
                        nc.tensor.matmul(
                            qph[nch],
                            lhsT=wqkv_sb[kc][:, 128 * qt:128 * (qt + 1)],
                            rhs=xc[kc][:, 512 * nch:512 * (nch + 1)],
                            start=(kc == 0), stop=(kc == CT - 1))
                eqh = []
                for nch in range(2):
                    eq = qpool.tile([128, HID], FP16, name="expq")
                    nc.scalar.activation(out=eq, in_=qph[nch], func=AF.Exp)
                    eqh.append(eq)
                if prev is not None:
                    emit_zq(*prev)
                prev = (qt, qph, eqh)
            emit_zq(*prev)

            # ---- final projection + bn stats ----
            for m in range(CT):
                fs = fpool.tile([128, NPIX], FP16, name="final")
                # pctx ring (idle after W2): keeps the pbig ring free so the
                # next batch's kv matmuls never wait on the final's PSUM
                # drain.  k4-outer so matmul pairs share the stationary W2
                # slice; both halves accumulate in parallel banks.
                fhs = [pctx.tile([128, HID], F32, name="fp", tag="ring")
                       for _ in range(2)]
                for k4 in range(HID // 128):
                    for nch in range(2):
                        nc.tensor.matmul(
                            fhs[nch],
                            lhsT=w2_sb[k4][:, 128 * m:128 * (m + 1)],
                            rhs=qhat[k4][:, 512 * nch:512 * (nch + 1)],
                            start=(k4 == 0), stop=(k4 == HID // 128 - 1))
                # stats read the PSUM halves directly and are emitted before
                # the casts so the cast never delays the stats chain that
                # feeds the AllReduce
                for nch in range(2):
                    nc.vector.bn_stats(
                        out=stats_sb[m][:, 2 * b + nch, :], in_=fhs[nch])
                if b == B - 1:
                    # defer the last batch's casts past the stats aggregation:
                    # they otherwise sit between the final bn_stats and the
                    # aggregate ops in the vector queue, delaying the
                    # AllReduce trigger by ~1.4us
                    deferred_cast.append((fs, fhs))
                else:
                    for nch in range(2):
                        nc.vector.tensor_copy(
                            fs[:, 512 * nch:512 * (nch + 1)], fhs[nch])
                final_sb[b][m] = fs

        # ---- batch-norm: aggregate, all-reduce, normalize, store ----
        ccin = dpool.tile([128, 2 * CT], F32, name="ccin")
        # Shared address space: HBM-HBM collectives with Shared outputs skip
        # a staging hop in the ncfw path (concourse warns about this for
        # large tensors; the latency also matters here)
        ccout = dpool.tile([128, 2 * CT], F32, name="ccout",
                           addr_space="Shared")
        no_cc = os.environ.get("BASS_ATTN_NO_CC") == "1"  # timing-only builds
        # switch the ACT table to the sqrt set while PE still runs the last
        # final-proj matmuls, so the tail's Sqrt doesn't pay the ~1.3us load.
        # Reading the last batch's final tile (not a constant) keeps the
        # scheduler from hoisting this to the start, which would thrash the
        # table back and forth around the first Exp.
        warm_sq = small.tile([1, 1], F32, name="warmsq")
        nc.scalar.activation(out=warm_sq, in_=final_sb[B - 1][CT - 1][0:1, 0:1],
                             func=AF.Sqrt)
        # stat-major packing: cols [mean_m0, mean_m1, ex2_m0, ex2_m1]
        pk4 = small.tile([128, 2 * CT], F32, name="pk4")
        for m in range(CT):
            mv = small.tile([128, 2], F32, name="mv")
            nc.vector.bn_aggr(out=mv, in_=stats_sb[m])
            nc.vector.tensor_mul(pk4[:, CT + m:CT + m + 1],
                                 mv[:, 0:1], mv[:, 0:1])
            nc.vector.tensor_add(pk4[:, CT + m:CT + m + 1],
                                 pk4[:, CT + m:CT + m + 1], mv[:, 1:2])
            nc.vector.tensor_copy(pk4[:, m:m + 1], mv[:, 0:1])
        nc.vector.tensor_scalar_mul(pk4, in0=pk4, scalar1=1.0 / N_CORES)
        nc.sync.dma_start(out=ccin, in_=pk4)
        for fs, fhs in deferred_cast:
            for nch in range(2):
                nc.vector.tensor_copy(
                    fs[:, 512 * nch:512 * (nch + 1)], fhs[nch])
        if not no_cc:
            nc.gpsimd.collective_compute(
                "AllReduce", ALU.add,
                replica_groups=[list(range(N_CORES))],
                ins=[ccin.opt()], outs=[ccout.opt()])
        gst = small.tile([128, 2 * CT], F32, name="gst")
        nc.sync.dma_start(out=gst, in_=ccout if not no_cc else ccin)
        gmean = gst[:, 0:CT]
        var = small.tile([128, CT], F32, name="var")
        nc.vector.tensor_mul(var, gmean, gmean)
        nc.vector.tensor_sub(var, gst[:, CT:2 * CT], var)
        std = small.tile([128, CT], F32, name="std")
        nc.scalar.activation(out=std, in_=var, func=AF.Sqrt, bias=eps_sb)
        rstd = small.tile([128, CT], F32, name="rstd")
        nc.vector.reciprocal_approx_fast(out=rstd, in_=std)
        rsg = small.tile([128, CT], F32, name="rsg")
        nc.vector.tensor_mul(rsg, rstd, gamma2)
        sh = small.tile([128, CT], F32, name="sh")
        nc.vector.tensor_mul(sh, gmean, rsg)
        nc.vector.tensor_sub(sh, beta2, sh)
        # apply per 512-col half, alternating vector/scalar, and store each
        # half with its own dma_start: every dma_start lands on its own ring
        # (~30 GB/s each), so 16x256KB transfers drain the 4MB output across
        # rings far faster than 8x512KB would
        idx = 0
        for b in range(B):
            for m in range(CT):
                fs = final_sb[b][m]
                yst = stg.tile([128, NPIX], F32, name="yst")
                for nch in range(2):
                    sl = slice(512 * nch, 512 * (nch + 1))
                    if idx % 2 == 0:
                        nc.vector.tensor_scalar(
                            out=yst[:, sl], in0=fs[:, sl],
                            scalar1=rsg[:, m:m + 1], scalar2=sh[:, m:m + 1],
                            op0=ALU.mult, op1=ALU.add)
                        nc.sync.dma_start(
                            out=y[b, 128 * m:128 * (m + 1), sl],
                            in_=yst[:, sl])
                    else:
                        nc.scalar.activation(
                            out=yst[:, sl], in_=fs[:, sl], func=AF.Identity,
                            bias=sh[:, m:m + 1], scale=rsg[:, m:m + 1])
                        nc.scalar.dma_start(
                            out=y[b, 128 * m:128 * (m + 1), sl],
                            in_=yst[:, sl])
                    idx += 1


_CACHE = {}


def _build():
    if "nc" in _CACHE:
        return _CACHE["nc"]
    nc = bacc.Bacc("TRN2", target_bir_lowering=False, debug=False,
                   enable_asserts=True, num_devices=N_CORES)
    x = nc.dram_tensor("x", [B, C, NPIX], FP16, kind="ExternalInput").ap()
    wqkv = nc.dram_tensor("wqkvT", [C, 3 * HID], FP16,
                          kind="ExternalInput").ap()
    wout = nc.dram_tensor("woutT", [HID, C], FP16, kind="ExternalInput").ap()
    ident = nc.dram_tensor("ident", [128, D], FP16, kind="ExternalInput").ap()
    gammab = nc.dram_tensor("gammab", [C, 1], F32, kind="ExternalInput").ap()
    betab = nc.dram_tensor("betab", [C, 1], F32, kind="ExternalInput").ap()
    y = nc.dram_tensor("y", [B, C, NPIX], F32, kind="ExternalOutput").ap()
    with tile.TileContext(nc) as tc:
        _emit(tc, x, wqkv, wout, ident, gammab, betab, y)
    nc.compile()
    _CACHE["nc"] = nc
    return nc


def kernel(x, w_qkv, w_out, b_out, gamma, beta, _trace=False):
    x = np.asarray(x, dtype=np.float32)
    wqkvT = np.ascontiguousarray(np.asarray(w_qkv, np.float16).T)   # [256, 1536]
    woutT = np.ascontiguousarray(np.asarray(w_out, np.float16).T)   # [512, 256]
    gammab = np.ascontiguousarray(np.asarray(gamma, np.float32).reshape(C, 1))
    betab = np.ascontiguousarray(np.asarray(beta, np.float32).reshape(C, 1))
    ident = np.zeros((128, D), dtype=np.float16)
    for p in range(128):
        ident[p, p % D] = 1.0
    # b_out is intentionally unused: BatchNorm's mean subtraction cancels any
    # per-channel constant added before it, exactly.

    btot, c, hh, ww = x.shape
    assert (btot, c, hh * ww) == (B * N_CORES, C, NPIX)
    xf = x.reshape(btot, C, NPIX)

    nc = _build()
    in_maps = []
    for core in range(N_CORES):
        in_maps.append({
            "x": np.ascontiguousarray(xf[B * core:B * (core + 1)]).astype(np.float16),
            "wqkvT": wqkvT,
            "woutT": woutT,
            "ident": ident,
            "gammab": gammab,
            "betab": betab,
        })
    res = run_bass_kernel_spmd(nc, in_maps, core_ids=list(range(N_CORES)),
                               trace=_trace)
    y = np.concatenate([res.results[core]["y"] for core in range(N_CORES)],
                       axis=0)
    out = y.reshape(btot, C, hh, ww).astype(np.float32)
    if _trace:
        kernel.last_result = res
    return out
